# revision 2
# baseline (speedup 1.0000x reference)
"""Trainium2 Bass kernel for the FFSNN v2 problem — v2 (engine-balanced).

Math (per timestep t, reference semantics):
    m_l = m_l * 0.2 * (1 - s_l) + inp @ W_l.T + b_l ;  s_l = (m_l > 0.5)
    acc += s3 @ W4.T + b4 ;  out = acc/T                      (T = 196)

v2 design (vs the v1 all-DVE baseline at ~5.6us/step):
  * sigma-convention spikes: sigma = Sign(0.5 - m) in {-1,+1} bf16, computed
    on the ACTIVATION engine (exact on HW, probe-verified). W@s with
    s=(1-sigma)/2 becomes [b + rowsum(W)/2] - (W/2)@sigma, so the matmul
    stationary is W/2 (bf16 hi+lo for ~fp32 accuracy) and the constant
    folds into the custom-op bias.
  * One fused custom DVE op per layer-chunk per step (runtime-registered
    uop table row, exactness probe-verified):
        out = select(m < 0.5, 0.2*m, 0) + (b'' - P)
    i.e. the entire membrane update (decay mask + bias + PSUM consume) in a
    single DVE instruction. DVE load drops ~5.9us -> ~3.0us/step.
  * Matmuls batched 2 timesteps wide (N=256 moving operand): halves PE SEQ
    and fixed overheads. PSUM: P2/P3 group tiles [128,1024] (2 banks each),
    double-buffered = all 8 banks.
  * Layer 1 (K=4) hoisted: negdelta[idx] = -(x_slice@W1.T + b1) precomputed
    on device with the 6-pass exact bf16^3 scheme (49 distinct slices);
    layer-1 runs 2 groups ahead of layer 2 on the same fused op.
  * acc accumulates sigma3 (bf16 ints, exact); readout folds
    sum(s3) = (T - sum(sigma3))/2 into one final matmul + affine.

Sharding: pure data parallel over 8 NeuronCores (batch 1024 -> 128/core),
weights replicated, no collectives.
"""

import os
import sys

sys.path.insert(0, "/opt/trn_rl_repo")

import numpy as np
import ml_dtypes

BF16 = ml_dtypes.bfloat16
F32 = np.float32

NCORES = 8
B = 1024
BC = B // NCORES          # 128 batch per core
T = 196
H = 512
NJ = 4                    # hidden chunks of 128
NIDX = 49                 # distinct layer-1 input slices
XG_N = NIDX * BC          # 6272
PRE_BLKS = 13             # ceil(6272/512); last block is 128 wide

_BUILT = None
LAST_EXEC_NS = None
LAST_RESULTS = None
_REPEAT = 1               # benchmarking knob: run the time loop N times

_SNN_OP = None


def _get_snn_op():
    """Runtime-register the fused membrane-update DVE op:
        out = select(in1 < s0, in1*imm2, 0) + (s1 - in0)
    in1 = m (state), in0 = P (PSUM), s0 = 0.5, s1 = per-partition bias,
    imm2 = 0.2. Exactness verified on HW (probe: maxerr 0.0)."""
    global _SNN_OP
    if _SNN_OP is not None:
        return _SNN_OP
    from concourse import dve_ops
    from concourse.dve_spec import C0, C1, C2, Spec, Src0, Src1, Zero, select, lower
    from concourse.dve_ops import DveOp
    from concourse.dve_uop import DveOpSpec

    name = "SNN_FUSED_ANT"
    spec = Spec(
        body=select(Src1 < C0, Src1 * C2, Zero) + (C1 - Src0),
        reference=lambda in0, in1, s0, s1, imm2: (
            np.where(in1 < s0, in1 * np.float32(imm2), np.float32(0.0))
            + (s1 - in0)
        ).astype(np.float32),
    )
    for op in dve_ops.OPS:
        if op.name == name:
            _SNN_OP = op
            return op
    row = max(dve_ops._SUB_OPCODE_FOR_NAME.values()) + 1
    assert row < 0x20
    dve_ops._SUB_OPCODE_FOR_NAME[name] = row
    shas = {}
    for ver in ("v3", "v4"):
        s = DveOpSpec(name=name, opcode=row, uops=lower(spec, ver=ver), rd1_en=True)
        shas[ver] = s.sha(ver)
    op = DveOp(name, spec, subdim=False, uops_sha=shas)
    dve_ops.OPS.append(op)
    dve_ops.CUSTOM_DVE_SPECS[name] = spec
    _SNN_OP = op
    return op


def _split(a):
    hi = a.astype(BF16)
    lo = (a.astype(F32) - hi.astype(F32)).astype(BF16)
    return hi, lo


def _split3(a):
    """Exact 3-way bf16 decomposition (24 mantissa bits = 3x8)."""
    p0 = a.astype(BF16)
    r = a.astype(F32) - p0.astype(F32)
    p1 = r.astype(BF16)
    p2 = (r - p1.astype(F32)).astype(BF16)
    return p0, p1, p2


def _prep_host(inputs):
    x = np.ascontiguousarray(inputs["x"], dtype=F32)          # [1024, 784]
    W1 = np.asarray(inputs["W1"], F32); b1 = np.asarray(inputs["b1"], F32)
    W2 = np.asarray(inputs["W2"], F32); b2 = np.asarray(inputs["b2"], F32)
    W3 = np.asarray(inputs["W3"], F32); b3 = np.asarray(inputs["b3"], F32)
    W4 = np.asarray(inputs["W4"], F32); b4 = np.asarray(inputs["b4"], F32)

    # gathered input, aug with ones row: xg[c, k, idx*BC + b] = x[c*BC+b, base+k]
    bases = [4 * i for i in range(48)] + [780]
    xg = np.empty((NCORES, 5, XG_N), F32)
    for i, s in enumerate(bases):
        blk = x[:, s:s + 4].reshape(NCORES, BC, 4).transpose(0, 2, 1)
        xg[:, 0:4, i * BC:(i + 1) * BC] = blk
    xg[:, 4, :] = 1.0
    x0, x1, x2 = _split3(xg)                                   # [8, 5, 6272]

    # negated so the fused op's (s1 - Src0) with s1=0 yields +delta
    W1a = -np.concatenate([W1.T, b1[None, :]], axis=0)         # [5, 512]
    w1p0, w1p1, w1p2 = _split3(W1a)

    def wlay(W):  # W/2 -> two bf16 [128, 2048] in (k,j)-block layout
        WT = (W.T * 0.5).astype(F32).copy()
        hi, lo = _split(WT)
        def lay(a):
            return np.ascontiguousarray(
                a.reshape(4, 128, 4, 128).transpose(1, 0, 2, 3).reshape(128, 2048))
        return lay(hi), lay(lo)

    w2h, w2l = wlay(W2)
    w3h, w3l = wlay(W3)

    W4T = (W4.T * 0.5).astype(F32).copy()                      # [512, 10]
    h4, l4 = _split(W4T)
    def lay4(a):
        return np.ascontiguousarray(a.reshape(4, 128, 10).transpose(1, 0, 2).reshape(128, 40))
    w4h, w4l = lay4(h4), lay4(l4)

    f64 = np.float64
    b2p = (b2.astype(f64) + W2.astype(f64).sum(1) * 0.5).astype(F32)
    b3p = (b3.astype(f64) + W3.astype(f64).sum(1) * 0.5).astype(F32)
    outb = (b4.astype(f64) + W4.astype(f64).sum(1) * 0.5).astype(F32).reshape(10, 1)
    b2c = np.ascontiguousarray(b2p.reshape(4, 128).T)          # [128, 4]
    b3c = np.ascontiguousarray(b3p.reshape(4, 128).T)

    shared = dict(w1p0=w1p0, w1p1=w1p1, w1p2=w1p2, w2h=w2h, w2l=w2l,
                  w3h=w3h, w3l=w3l, w4h=w4h, w4l=w4l, b2c=b2c, b3c=b3c,
                  outb=outb)
    in_maps = []
    for c in range(NCORES):
        m = dict(shared)
        m["x0"] = np.ascontiguousarray(x0[c])
        m["x1"] = np.ascontiguousarray(x1[c])
        m["x2"] = np.ascontiguousarray(x2[c])
        in_maps.append(m)
    return in_maps


def _build():
    import concourse.mybir as mybir
    import concourse.tile as tile
    from concourse import bacc

    dt = mybir.dt
    op = mybir.AluOpType
    AF = mybir.ActivationFunctionType
    snn = _get_snn_op()

    nc = bacc.Bacc()

    d_x0 = nc.dram_tensor("x0", [5, XG_N], dt.bfloat16, kind="ExternalInput")
    d_x1 = nc.dram_tensor("x1", [5, XG_N], dt.bfloat16, kind="ExternalInput")
    d_x2 = nc.dram_tensor("x2", [5, XG_N], dt.bfloat16, kind="ExternalInput")
    d_w1p0 = nc.dram_tensor("w1p0", [5, H], dt.bfloat16, kind="ExternalInput")
    d_w1p1 = nc.dram_tensor("w1p1", [5, H], dt.bfloat16, kind="ExternalInput")
    d_w1p2 = nc.dram_tensor("w1p2", [5, H], dt.bfloat16, kind="ExternalInput")
    d_w2h = nc.dram_tensor("w2h", [128, 2048], dt.bfloat16, kind="ExternalInput")
    d_w2l = nc.dram_tensor("w2l", [128, 2048], dt.bfloat16, kind="ExternalInput")
    d_w3h = nc.dram_tensor("w3h", [128, 2048], dt.bfloat16, kind="ExternalInput")
    d_w3l = nc.dram_tensor("w3l", [128, 2048], dt.bfloat16, kind="ExternalInput")
    d_w4h = nc.dram_tensor("w4h", [128, 40], dt.bfloat16, kind="ExternalInput")
    d_w4l = nc.dram_tensor("w4l", [128, 40], dt.bfloat16, kind="ExternalInput")
    d_b2c = nc.dram_tensor("b2c", [128, 4], dt.float32, kind="ExternalInput")
    d_b3c = nc.dram_tensor("b3c", [128, 4], dt.float32, kind="ExternalInput")
    d_outb = nc.dram_tensor("outb", [10, 1], dt.float32, kind="ExternalInput")
    d_y = nc.dram_tensor("y", [10, BC], dt.float32, kind="ExternalOutput")

    STEPS = T * _REPEAT
    GS = 2                    # steps per matmul group (N = GS*128 moving)
    G = STEPS // GS

    with tile.TileContext(nc) as tc:
        with tc.tile_pool(name="const", bufs=1) as cp:
            x0 = cp.tile([5, XG_N], dt.bfloat16)
            x1 = cp.tile([5, XG_N], dt.bfloat16)
            x2 = cp.tile([5, XG_N], dt.bfloat16)
            w1p0 = cp.tile([5, H], dt.bfloat16)
            w1p1 = cp.tile([5, H], dt.bfloat16)
            w1p2 = cp.tile([5, H], dt.bfloat16)
            w2h = cp.tile([128, 2048], dt.bfloat16)
            w2l = cp.tile([128, 2048], dt.bfloat16)
            w3h = cp.tile([128, 2048], dt.bfloat16)
            w3l = cp.tile([128, 2048], dt.bfloat16)
            w4h = cp.tile([128, 40], dt.bfloat16)
            w4l = cp.tile([128, 40], dt.bfloat16)
            b2c = cp.tile([128, 4], dt.float32)
            b3c = cp.tile([128, 4], dt.float32)
            outb = cp.tile([10, 1], dt.float32)
            for sb, dr in [(x0, d_x0), (x1, d_x1), (x2, d_x2), (w1p0, d_w1p0),
                           (w1p1, d_w1p1), (w1p2, d_w1p2),
                           (w2h, d_w2h), (w2l, d_w2l), (w3h, d_w3h), (w3l, d_w3l),
                           (w4h, d_w4h), (w4l, d_w4l), (b2c, d_b2c), (b3c, d_b3c),
                           (outb, d_outb)]:
                nc.sync.dma_start(sb, dr[:])

            negdelta = cp.tile([128, NIDX * H], dt.float32)    # 98 KB/partition
            half = cp.tile([128, 1], dt.float32)
            # membrane ping-pong: state(t) lives in [mB, mA][t % 2]
            m1a = cp.tile([128, H], dt.float32)
            m1b = cp.tile([128, H], dt.float32)
            m2a = cp.tile([128, H], dt.float32)
            m2b = cp.tile([128, H], dt.float32)
            m3a = cp.tile([128, H], dt.float32)
            m3b = cp.tile([128, H], dt.float32)
            accs = cp.tile([128, H], dt.bfloat16)
            y_sb = cp.tile([10, BC], dt.float32)

            nc.vector.memset(half, 0.5)
            nc.vector.memset(m1a, 0.0)
            nc.vector.memset(m2a, 0.0)
            nc.vector.memset(m3a, 0.0)
            nc.vector.memset(accs, 0.0)

            def m_of(tiles, t):
                # state AFTER update t; t = -1 is the zero init (the 'a' tile)
                return tiles[0] if (t % 2 == 0) else tiles[1]
            m1 = lambda t: m_of((m1b, m1a), t)
            m2 = lambda t: m_of((m2b, m2a), t)
            m3 = lambda t: m_of((m3b, m3a), t)

            # ---- layer-1 precompute: negdelta[idx] = -(x_slice @ W1.T + b1) ----
            d1v = negdelta.rearrange("p (i j b) -> p i j b", j=NJ, b=BC)
            with tc.tile_pool(name="ppre", bufs=4, space="PSUM") as ppre:
                for j in range(NJ):
                    for blk in range(PRE_BLKS):
                        n = 512 if blk < PRE_BLKS - 1 else XG_N - 512 * (PRE_BLKS - 1)
                        nq = n // BC
                        ps = ppre.tile([128, 512], dt.float32, tag="pre")
                        passes = [(w1p0, x0), (w1p0, x1), (w1p1, x0),
                                  (w1p1, x1), (w1p0, x2), (w1p2, x0)]
                        for pi, (wa, xa) in enumerate(passes):
                            nc.tensor.matmul(
                                ps[:, :n],
                                wa[:, j * 128:(j + 1) * 128],
                                xa[:, blk * 512:blk * 512 + n],
                                start=(pi == 0), stop=(pi == len(passes) - 1))
                        src = ps.rearrange("p (q b) -> p q b", b=BC)[:, :nq, :]
                        dst = d1v[:, 4 * blk:4 * blk + nq, j, :]
                        nc.scalar.activation(dst, src, AF.Copy)

            # ---- main time loop: GS-step groups, N = GS*128 matmuls ----
            NW = GS * BC          # moving free width per k-chunk
            with tc.tile_pool(name="sig1p", bufs=3) as sig1p, \
                 tc.tile_pool(name="sig2p", bufs=2) as sig2p, \
                 tc.tile_pool(name="sig3p", bufs=2) as sig3p, \
                 tc.tile_pool(name="pmm", bufs=2, space="PSUM") as pp:

                sig1_t = {}
                sig2_t = {}
                p2_t = {}
                p3_t = {}

                def sig_view(tile_, s):
                    v = tile_.rearrange("p (k s b) -> p k s b", k=NJ, s=GS, b=BC)
                    return v[:, :, s, :]

                def emit_l1(g):
                    sig = sig1p.tile([128, GS * H], dt.bfloat16, tag="sig1")
                    sig1_t[g] = sig
                    for s in range(GS):
                        t = GS * g + s
                        idx = min(t % T, 48)
                        nc.vector._custom_dve(
                            snn, out=m1(t),
                            in0=negdelta[:, idx * H:(idx + 1) * H],
                            in1=m1(t - 1), s0=0.5, s1=0.0, imm2=0.2)
                        nc.scalar.activation(sig_view(sig, s), m1(t), AF.Sign,
                                             bias=half[:, 0:1], scale=-1.0)

                def emit_mm(g, store, sigs, whi, wlo, tag):
                    P = pp.tile([128, NJ * NW], dt.float32, tag=tag)
                    store[g] = P
                    sig = sigs.pop(g)
                    for j in range(NJ):
                        for k in range(NJ):
                            for h, wt in enumerate((whi, wlo)):
                                nc.tensor.matmul(
                                    P[:, j * NW:(j + 1) * NW],
                                    wt[:, (k * NJ + j) * 128:(k * NJ + j + 1) * 128],
                                    sig[:, k * NW:(k + 1) * NW],
                                    start=(k == 0 and h == 0),
                                    stop=(k == NJ - 1 and h == 1))

                def emit_w2(g):
                    emit_mm(g, p2_t, sig1_t, w2h, w2l, "P2")

                def emit_w3(g):
                    emit_mm(g, p3_t, sig2_t, w3h, w3l, "P3")

                def emit_l2(g):
                    P2 = p2_t.pop(g)
                    sig = sig2p.tile([128, GS * H], dt.bfloat16, tag="sig2")
                    sig2_t[g] = sig
                    for s in range(GS):
                        t = GS * g + s
                        for j in range(NJ):
                            js = slice(j * 128, (j + 1) * 128)
                            nc.vector._custom_dve(
                                snn, out=m2(t)[:, js],
                                in0=P2[:, j * NW + s * 128:j * NW + (s + 1) * 128],
                                in1=m2(t - 1)[:, js],
                                s0=0.5, s1=b2c[:, j:j + 1], imm2=0.2)
                        nc.scalar.activation(sig_view(sig, s), m2(t), AF.Sign,
                                             bias=half[:, 0:1], scale=-1.0)

                def emit_l3(g):
                    P3 = p3_t.pop(g)
                    for s in range(GS):
                        t = GS * g + s
                        for j in range(NJ):
                            js = slice(j * 128, (j + 1) * 128)
                            nc.vector._custom_dve(
                                snn, out=m3(t)[:, js],
                                in0=P3[:, j * NW + s * 128:j * NW + (s + 1) * 128],
                                in1=m3(t - 1)[:, js],
                                s0=0.5, s1=b3c[:, j:j + 1], imm2=0.2)
                        s3 = sig3p.tile([128, H], dt.bfloat16, tag="sig3")
                        nc.scalar.activation(s3, m3(t), AF.Sign,
                                             bias=half[:, 0:1], scale=-1.0)
                        nc.vector.tensor_tensor(accs, accs, s3, op.add)

                # Emission order per slot g: PE [W2(g), W3(g-1)]; DVE/Act
                # [l1(g+2), l2(g), l3(g-1)].
                emit_l1(0)
                if G > 1:
                    emit_l1(1)
                for g in range(G):
                    emit_w2(g)
                    if g >= 1:
                        emit_w3(g - 1)
                    if g + 2 < G:
                        emit_l1(g + 2)
                    emit_l2(g)
                    if g >= 1:
                        emit_l3(g - 1)
                emit_w3(G - 1)
                emit_l3(G - 1)

                # ---- readout: y = outb - ((W4/2) @ accs)/T ----
                Pout = pp.tile([10, BC], dt.float32, tag="P3")
                for k in range(NJ):
                    for h, wt in enumerate((w4h, w4l)):
                        nc.tensor.matmul(
                            Pout, wt[:, k * 10:(k + 1) * 10],
                            accs[:, k * 128:(k + 1) * 128],
                            start=(k == 0 and h == 0),
                            stop=(k == NJ - 1 and h == 1))
                nc.scalar.activation(y_sb, Pout, AF.Identity,
                                     bias=outb[:, 0:1], scale=float(-1.0 / T))
                nc.sync.dma_start(d_y[:], y_sb)

    nc.finalize()
    return nc


def kernel(**inputs):
    global _BUILT, LAST_EXEC_NS, LAST_RESULTS
    from concourse import bass_utils

    in_maps = _prep_host(inputs)
    if _BUILT is None:
        _BUILT = _build()
    nc = _BUILT

    trace = bool(int(os.environ.get("KERNEL_TRACE", "0")))
    try:
        res = bass_utils.run_bass_kernel_spmd(
            nc, in_maps, core_ids=list(range(NCORES)), trace=trace)
    except ModuleNotFoundError:
        res = bass_utils.run_bass_kernel_spmd(
            nc, in_maps, core_ids=list(range(NCORES)), trace=False)
    LAST_EXEC_NS = res.exec_time_ns
    LAST_RESULTS = res

    out = np.empty((B, 10), F32)
    for c in range(NCORES):
        out[c * BC:(c + 1) * BC, :] = np.asarray(res.results[c]["y"]).T
    return out
